# revision 1
# baseline (speedup 1.0000x reference)
import numpy as np
import jax
import jax.numpy as jnp
from jax import lax

jax.config.update("jax_default_matmul_precision", "highest")

B, S, PAD, M1 = 256, 32, 2, 12
SP = S + PAD  # 34
NDEV = 8


def _dft_consts():
    w = np.arange(SP)
    k = np.arange(M1)
    ang = -2 * np.pi * np.outer(w, k) / SP          # [34,12] forward rfft cols 0..11
    Wwr, Wwi = np.cos(ang), np.sin(ang)
    rows = np.concatenate([np.arange(M1), np.arange(SP - M1, SP)])  # 0..11, 22..33
    angh = -2 * np.pi * np.outer(w, rows) / SP      # [34h, 24r]
    Ehr, Ehi = np.cos(angh), np.sin(angh)
    angih = 2 * np.pi * np.outer(rows, w) / SP      # [24r, 34p]
    Ghr, Ghi = np.cos(angih) / SP, np.sin(angih) / SP
    c = np.where(k == 0, 1.0, 2.0)[:, None]
    angiw = 2 * np.pi * np.outer(k, w) / SP         # [12k, 34q]
    Gwr, Gwi = c * np.cos(angiw) / SP, c * np.sin(angiw) / SP
    return [a.astype(np.float32) for a in (Wwr, Wwi, Ehr, Ehi, Ghr, Ghi, Gwr, Gwi)]


WWR, WWI, EHR, EHI, GHR, GHI, GWR, GWI = _dft_consts()


def _gelu(v):
    return jax.nn.gelu(v, approximate=False)


def _spectral(h, wr, wi):
    # h: [b,C,34,34]; wr/wi: [C,C,24,12] (top rows 0..11, bot rows 12..23)
    hwr = jnp.einsum('bchw,wk->bchk', h, WWR)
    hwi = jnp.einsum('bchw,wk->bchk', h, WWI)
    zr = jnp.einsum('bchk,hr->bcrk', hwr, EHR) - jnp.einsum('bchk,hr->bcrk', hwi, EHI)
    zi = jnp.einsum('bchk,hr->bcrk', hwr, EHI) + jnp.einsum('bchk,hr->bcrk', hwi, EHR)
    tr = jnp.einsum('birk,iork->bork', zr, wr) - jnp.einsum('birk,iork->bork', zi, wi)
    ti = jnp.einsum('birk,iork->bork', zr, wi) + jnp.einsum('birk,iork->bork', zi, wr)
    yr = jnp.einsum('bork,rp->bopk', tr, GHR) - jnp.einsum('bork,rp->bopk', ti, GHI)
    yi = jnp.einsum('bork,rp->bopk', tr, GHI) + jnp.einsum('bork,rp->bopk', ti, GHR)
    x1 = jnp.einsum('bopk,kq->bopq', yr, GWR) - jnp.einsum('bopk,kq->bopq', yi, GWI)
    return x1


def _forward(x, grid, sentence_embeddings, fc0_w, fc0_b, sc_w1, sc_w2, wc_w, wc_b,
             pe1_w, pe1_b, pe2_w, pe2_b,
             sp_w1, sp_b1, sp_w2, sp_b2, sp_w3, sp_b3,
             xp_w1, xp_b1, xp_w2, xp_b2, xp_w3, xp_b3,
             pu_w1, pu_b1, pu_w2, pu_b2, pu_w3, pu_b3,
             fc1_w, fc1_b, fc2_w, fc2_b):
    b = x.shape[0]
    s = jax.nn.relu(sentence_embeddings @ sp_w1 + sp_b1)
    s = jax.nn.relu(s @ sp_w2 + sp_b2)
    sentence_emb = s @ sp_w3 + sp_b3  # [b,16]

    h = jnp.concatenate([x, grid], axis=-1) @ fc0_w + fc0_b  # [b,32,32,C]
    h = h.transpose(0, 3, 1, 2)
    h = jnp.pad(h, ((0, 0), (0, 0), (0, PAD), (0, PAD)))  # [b,C,34,34]

    for i in range(4):
        # stack top(w1)/bot(w2) along the r axis -> [C,C,24,12]
        wr = jnp.concatenate([sc_w1[i, ..., 0], sc_w2[i, ..., 0]], axis=2)
        wi = jnp.concatenate([sc_w1[i, ..., 1], sc_w2[i, ..., 1]], axis=2)
        x1 = _spectral(h, wr, wi)
        x2 = jnp.einsum('bchw,oc->bohw', h, wc_w[i]) + wc_b[i][None, :, None, None]
        h = x1 + x2
        if i < 3:
            h = _gelu(h)

    p = lax.conv_general_dilated(h, pe1_w, (4, 4), 'VALID',
                                 dimension_numbers=('NCHW', 'OIHW', 'NCHW'))
    p = _gelu(p + pe1_b[None, :, None, None])  # [b,1,7,7]
    p = jnp.einsum('bchw,oc->bohw', p, pe2_w) + pe2_b[None, :, None, None]
    p = p.reshape(b, -1)  # [b,49]

    e = jax.nn.silu(p @ xp_w1 + xp_b1)
    e = jax.nn.silu(e @ xp_w2 + xp_b2)
    x_emb = e @ xp_w3 + xp_b3  # [b,16]

    emb = jnp.concatenate([x_emb, sentence_emb], axis=-1)
    emb = jax.nn.silu(emb @ pu_w1 + pu_b1)
    emb = jax.nn.silu(emb @ pu_w2 + pu_b2)
    emb = (emb @ pu_w3 + pu_b3).reshape(b, 1, SP, SP)

    h = jnp.concatenate([h, emb], axis=1)  # [b,C+1,34,34]
    h = h[..., :-PAD, :-PAD]
    h = h.transpose(0, 2, 3, 1)
    h = _gelu(h @ fc1_w + fc1_b)
    out = h @ fc2_w + fc2_b
    return out[..., None, :]


_ORDER = ['x', 'grid', 'sentence_embeddings', 'fc0_w', 'fc0_b', 'sc_w1', 'sc_w2',
          'wc_w', 'wc_b', 'pe1_w', 'pe1_b', 'pe2_w', 'pe2_b',
          'sp_w1', 'sp_b1', 'sp_w2', 'sp_b2', 'sp_w3', 'sp_b3',
          'xp_w1', 'xp_b1', 'xp_w2', 'xp_b2', 'xp_w3', 'xp_b3',
          'pu_w1', 'pu_b1', 'pu_w2', 'pu_b2', 'pu_w3', 'pu_b3',
          'fc1_w', 'fc1_b', 'fc2_w', 'fc2_b']

_PMAP = None


def _get_pmap(ndev):
    global _PMAP
    if _PMAP is None:
        in_axes = (0, 0, 0) + (None,) * (len(_ORDER) - 3)
        _PMAP = jax.pmap(_forward, in_axes=in_axes, out_axes=0)
    return _PMAP


def kernel(**inputs):
    args = [np.asarray(inputs[n]) for n in _ORDER]
    devs = jax.devices()
    ndev = NDEV if len(devs) >= NDEV else 1
    if ndev > 1:
        sh = [a.reshape((ndev, a.shape[0] // ndev) + a.shape[1:]) for a in args[:3]]
        fn = _get_pmap(ndev)
        out = fn(*sh, *args[3:])
        out = np.asarray(out)
        out = out.reshape((out.shape[0] * out.shape[1],) + out.shape[2:])
    else:
        out = np.asarray(jax.jit(_forward)(*args))
    return out.astype(np.float32)
